# revision 1
# baseline (speedup 1.0000x reference)
"""GAT (3-layer DGL-style) on 8 Trainium2 NeuronCores.

Sharding: nodes partitioned contiguously across 8 cores (6250 each, relabeled
within each core by in-degree for slot-grid uniformity). Edges sharded by dst
core. Per layer: dense matmul (bf16) produces per-node rows [h | el] plus er
kept in SBUF; AllGather replicates the row table; each core then runs the
edge phase for its own dsts: per group-of-blocks dma_gather (2 gathers:
pass A/B over the int16-index split), batched 4D/5D DVE ops for the
softmax + weighted aggregation in a [dst-partition x slot] layout.
"""

import numpy as np
import ml_dtypes

import concourse.bacc as bacc
import concourse.bass as bass
import concourse.mybir as mybir
from concourse import tile
from concourse._compat import cdiv
from concourse.bass_utils import run_bass_kernel_spmd
from bass_rust import SemaphoreHandle

N = 50000
E = 800000
NC = 8
L = N // NC              # 6250 nodes per core
NBLK = cdiv(L, 128)      # 49 dst blocks per core
HEADS = 4
HD = 32
HID = 128
OUT = 64
F0 = 256
NEG = 0.2
ABOUND = 5 * L           # nodes with new id < ABOUND are "pass A" (31250)
GROUP_COLS = 64          # slot-column budget per gather group
GROUP_MAXB = 6           # max blocks per gather group
import os as _os_pad
_PADIDX = -1 if _os_pad.environ.get("GAT_NEGPAD") else 0
_NODVE = bool(_os_pad.environ.get("GAT_NODVE"))

F32 = mybir.dt.float32
BF16 = mybir.dt.bfloat16
I16 = mybir.dt.int16
AF = mybir.ActivationFunctionType
OP = mybir.AluOpType
AX = mybir.AxisListType


def _split_multiwaits(nc):
    nsplit = 0
    for bb in nc.main_func.blocks:
        i = 0
        while i < len(bb.instructions):
            ins = bb.instructions[i]
            si = ins.sync_info
            if si is not None and si.on_wait and len(si.on_wait) > 1:
                waits = list(si.on_wait)
                new_insts = []
                for w in waits[:-1]:
                    h = SemaphoreHandle(name=w.ant_name, num=w.id)
                    eng = nc.engines[ins.engine]
                    if w.wait_mode == "sem-ge-imm":
                        wi = eng.wait_ge(h, w.wait_value)
                    elif w.wait_mode == "sem-eq-imm":
                        wi = eng.wait_op(h, w.wait_value, "==")
                    else:
                        raise AssertionError(w.wait_mode)
                    removed = False
                    for b2 in nc.main_func.blocks:
                        if b2.instructions and b2.instructions[-1].name == wi.ins.name:
                            b2.instructions.pop()
                            removed = True
                            break
                    assert removed
                    new_insts.append(wi.ins)
                si.on_wait = [waits[-1]]
                for k, n in enumerate(new_insts):
                    bb.instructions.insert(i + k, n)
                i += len(new_insts)
                nsplit += 1
            i += 1
    return nsplit


def _cumcount(groups):
    """j-th occurrence index within each group (groups sorted)."""
    n = len(groups)
    if n == 0:
        return np.zeros(0, np.int64)
    first = np.r_[True, groups[1:] != groups[:-1]]
    idx = np.arange(n)
    start = idx[first]
    return idx - np.repeat(start, np.diff(np.r_[idx[first], n]))


def _wrap_idx(flat):
    """[nidx] stream -> [128, nidx//16] int16 wrapped index tile."""
    nidx = flat.shape[0]
    assert nidx % 128 == 0
    S = nidx // 16
    t = flat.reshape(S, 16).T.astype(np.int16)   # [16, S]
    return np.tile(t, (8, 1))                    # [128, S]


def _preprocess(src, dst):
    src = np.asarray(src, np.int64)
    dst = np.asarray(dst, np.int64)
    half = (src // L) >= 5          # pass B edges (src in cores 5-7)

    degA = np.bincount(dst[~half], minlength=N)
    degB = np.bincount(dst[half], minlength=N)

    perm = np.empty(N, np.int64)        # old id -> new id
    node_order = np.empty(N, np.int64)  # new id -> old id
    for c in range(NC):
        nodes = np.arange(c * L, (c + 1) * L)
        order = np.lexsort((-degB[nodes], -degA[nodes]))
        node_order[c * L : (c + 1) * L] = nodes[order]
        perm[nodes[order]] = c * L + np.arange(L)

    nsrc = perm[src]
    ndst = perm[dst]
    epass = (nsrc >= ABOUND).astype(np.int64)

    cntA = np.bincount(ndst[epass == 0], minlength=N)
    cntB = np.bincount(ndst[epass == 1], minlength=N)

    # program-level W per (block, pass): max over cores
    WA = np.zeros(NBLK, np.int64)
    WB = np.zeros(NBLK, np.int64)
    for c in range(NC):
        la = np.zeros(NBLK * 128, np.int64)
        lb = np.zeros(NBLK * 128, np.int64)
        la[:L] = cntA[c * L : (c + 1) * L]
        lb[:L] = cntB[c * L : (c + 1) * L]
        WA = np.maximum(WA, la.reshape(NBLK, 128).max(1))
        WB = np.maximum(WB, lb.reshape(NBLK, 128).max(1))

    # adaptive grouping: uniform per-group VIEW widths (bounded footprint);
    # gathers stay tight per (block, pass) — padded view columns are masked.
    groups = []  # (b0, nb, WAg, WBg)
    b = 0
    while b < NBLK:
        nb = 1
        wag, wbg = int(WA[b]), int(WB[b])
        while b + nb < NBLK and nb < GROUP_MAXB:
            nwa = max(wag, int(WA[b + nb]))
            nwb = max(wbg, int(WB[b + nb]))
            if (nb + 1) * (nwa + nwb) > GROUP_COLS and nb >= 1:
                break
            wag, wbg = nwa, nwb
            nb += 1
        groups.append((b, nb, wag, wbg))
        b += nb

    # mask columns: group-major, block-major within group, [A slots | B slots]
    moffs = []
    Wtot = 0
    for (b0, nb, wag, wbg) in groups:
        moffs.append(Wtot)
        Wtot += nb * (wag + wbg)

    # idx stream offsets: tight per-(block, pass) pieces, A blocks then B
    soffs = []      # per group: start col
    boffs = []      # per group: per-block (offA, offB) within the group stream
    S16tot = 0
    for (b0, nb, wag, wbg) in groups:
        soffs.append(S16tot)
        per = []
        off = 0
        for bi in range(nb):
            per.append([off, 0])
            off += 8 * int(WA[b0 + bi])
        for bi in range(nb):
            per[bi][1] = off
            off += 8 * int(WB[b0 + bi])
        boffs.append([tuple(x) for x in per])
        S16tot += off

    idx_alls = []
    msk_alls = []
    for c in range(NC):
        m = (ndst // L) == c
        es = nsrc[m]
        ed = ndst[m] - c * L
        eq = epass[m]
        okey = ed * 2 + eq
        order = np.argsort(okey, kind="stable")
        es, ed, eq = es[order], ed[order], eq[order]
        j = _cumcount(okey[order])
        blk = ed // 128
        p = ed % 128

        msk = np.zeros((128, Wtot), np.float32)
        idx_pieces = []
        for gi, (b0, nb, wag, wbg) in enumerate(groups):
            for q, wq in ((0, WA), (1, WB)):
                for bi in range(nb):
                    w = int(wq[b0 + bi])
                    if w == 0:
                        continue
                    sel = (blk == b0 + bi) & (eq == q)
                    grid = np.zeros((128, w), np.int64)
                    v = es[sel] - (ABOUND if q else 0)
                    grid[p[sel], j[sel]] = v
                    idx_pieces.append(_wrap_idx(grid.T.reshape(-1)))

            mo = moffs[gi]
            W = wag + wbg
            in_g = (blk >= b0) & (blk < b0 + nb)
            bi_g = blk[in_g] - b0
            pg = p[in_g]
            jg = j[in_g]
            qg = eq[in_g]
            mgrid = msk[:, mo : mo + nb * W].reshape(128, nb, W)
            selA = qg == 0
            mgrid[pg[selA], bi_g[selA], jg[selA]] = 1.0
            selB = ~selA
            mgrid[pg[selB], bi_g[selB], wag + jg[selB]] = 1.0

        idx_alls.append(np.concatenate(idx_pieces, axis=1))
        msk_alls.append(msk.astype(ml_dtypes.bfloat16))

    assert idx_alls[0].shape[1] == S16tot
    meta = dict(groups=groups, moffs=moffs, soffs=soffs, boffs=boffs,
                WA=WA, WB=WB, Wtot=Wtot,
                S16tot=S16tot, node_order=node_order, perm=perm)
    return meta, idx_alls, msk_alls


def _weights_ext(W, al, ar, heads, hd):
    K = W.shape[0]
    Wr = W.reshape(K, heads, hd)
    A = np.einsum("khd,hd->kh", Wr, al).astype(np.float32)
    B = np.einsum("khd,hd->kh", Wr, ar).astype(np.float32)
    We = np.concatenate([W, A, B], axis=1).astype(np.float32)
    pad = (-We.shape[1]) % 4
    if pad:
        We = np.concatenate([We, np.zeros((K, pad), np.float32)], axis=1)
    return We.astype(ml_dtypes.bfloat16)


def _build_program(meta):
    import os as _os0
    _SP = bool(_os0.environ.get("GAT_SP"))
    groups = meta["groups"]
    moffs = meta["moffs"]
    soffs = meta["soffs"]
    boffs = meta["boffs"]
    WAv, WBv = meta["WA"], meta["WB"]
    S16tot = meta["S16tot"]
    Wtot = meta["Wtot"]
    MGCAP = max(nb * (wag + wbg) for (_b0, nb, wag, wbg) in groups)

    nc = bacc.Bacc("TRN2", num_swdge_queues=4)
    LP = NBLK * 128  # padded node count per core (6272)

    featT = nc.dram_tensor("featT", [F0, L], BF16, kind="ExternalInput")
    W1e = nc.dram_tensor("W1e", [F0, 136], BF16, kind="ExternalInput")
    W2e = nc.dram_tensor("W2e", [HID, 136], BF16, kind="ExternalInput")
    W3e = nc.dram_tensor("W3e", [HID, 68], BF16, kind="ExternalInput")
    al1r = nc.dram_tensor("al1r", [128, HID], BF16, kind="ExternalInput")
    al2r = nc.dram_tensor("al2r", [128, HID], BF16, kind="ExternalInput")
    b1r = nc.dram_tensor("b1r", [128, HID], F32, kind="ExternalInput")
    b2r = nc.dram_tensor("b2r", [128, HID], F32, kind="ExternalInput")
    b3r = nc.dram_tensor("b3r", [128, OUT], F32, kind="ExternalInput")
    ident_in = nc.dram_tensor("ident", [128, 128], F32, kind="ExternalInput")
    idx_in = nc.dram_tensor("idx_all", [128, S16tot], I16, kind="ExternalInput")
    msk_in = nc.dram_tensor("msk_all", [128, Wtot], BF16, kind="ExternalInput")
    out_ext = nc.dram_tensor("out", [LP, OUT], F32, kind="ExternalOutput")

    import os as _os
    DBG = bool(_os.environ.get("GAT_DEBUG"))
    dbg_t = {}
    if DBG:
        b0d, nbd, wagd, wbgd = groups[0]
        Wd = wagd + wbgd
        for li in range(3):
            hh = 1 if li == 2 else HEADS
            hdd = OUT if li == 2 else HD
            dbg_t[(li, "lg")] = nc.dram_tensor(f"dbg_lg{li}", [128, nbd * Wd * hh], F32, kind="ExternalOutput")
            dbg_t[(li, "ex")] = nc.dram_tensor(f"dbg_ex{li}", [128, nbd * Wd * hh], F32, kind="ExternalOutput")
            dbg_t[(li, "den")] = nc.dram_tensor(f"dbg_den{li}", [128, nbd * hh], F32, kind="ExternalOutput")
            dbg_t[(li, "acc")] = nc.dram_tensor(f"dbg_acc{li}", [128, nbd * hh * hdd], F32, kind="ExternalOutput")
            dbg_t[(li, "mgel")] = nc.dram_tensor(f"dbg_mgel{li}", [128, nbd * Wd * hh], F32, kind="ExternalOutput")
            dbg_t[(li, "tmp")] = nc.dram_tensor(f"dbg_tmp{li}", [128, nbd * hh * hdd * Wd], BF16, kind="ExternalOutput")

    ROW12, ROW3 = 128, 128
    tab_loc1 = nc.dram_tensor("tab_loc1", [L, ROW12], BF16)
    tab_loc2 = nc.dram_tensor("tab_loc2", [L, ROW12], BF16)
    tab_loc3 = nc.dram_tensor("tab_loc3", [L, ROW3], BF16)
    tab1 = nc.dram_tensor("tab1", [N, ROW12], BF16, addr_space="Shared")
    tab2 = nc.dram_tensor("tab2", [N, ROW12], BF16, addr_space="Shared")
    tab3 = nc.dram_tensor("tab3", [N, ROW3], BF16, addr_space="Shared")

    layers = [
        dict(Fin=F0, Fout=HID, heads=HEADS, hd=HD, W=W1e, ncols=136, row=ROW12,
             tloc=tab_loc1, tfull=tab1, brep=b1r, relu=True, alr=al1r),
        dict(Fin=HID, Fout=HID, heads=HEADS, hd=HD, W=W2e, ncols=136, row=ROW12,
             tloc=tab_loc2, tfull=tab2, brep=b2r, relu=True, alr=al2r),
        dict(Fin=HID, Fout=OUT, heads=1, hd=OUT, W=W3e, ncols=68, row=ROW3,
             tloc=tab_loc3, tfull=tab3, brep=b3r, relu=False, alr=None),
    ]

    with tile.TileContext(nc) as tc:
        with (
            tc.tile_pool(name="persist", bufs=1) as pp,
            tc.tile_pool(name="work", bufs=2) as wp,
            tc.tile_pool(name="soft", bufs=2) as sp,
            tc.tile_pool(name="idxp", bufs=3) as ixp,
            tc.tile_pool(name="tmp", bufs=1) as tp,
            tc.tile_pool(name="psum", bufs=2, space="PSUM") as psp,
            tc.tile_pool(name="psumT", bufs=2, space="PSUM") as pspT,
        ):
            idx_sb = pp.tile([128, S16tot], I16, tag="idx")
            nc.sync.dma_start(idx_sb[:], idx_in[:])
            msk_sb = pp.tile([128, Wtot], BF16, tag="msk")
            nc.sync.dma_start(msk_sb[:], msk_in[:])
            ident = pp.tile([128, 128], F32, tag="ident")
            nc.sync.dma_start(ident[:], ident_in[:])

            # xT buffers (features x nodes), bf16
            xT_a0 = pp.tile([128, LP], BF16, tag="xTa0")
            xT_a1 = pp.tile([128, LP], BF16, tag="xTa1")  # 2nd K-tile (layer 0)
            xT_b = pp.tile([128, LP], BF16, tag="xTb")
            nc.sync.dma_start(xT_a0[:, 0:L], featT[0:128, :])
            nc.sync.dma_start(xT_a1[:, 0:L], featT[128:256, :])

            _gq = [0]  # gather queue round-robin counter
            er_all = pp.tile([128, NBLK, HEADS], F32, tag="er")
            nc.vector.memset(er_all[:], 0.0)
            bias_sb = pp.tile([128, HID], F32, tag="bias")

            # persistent gather buffers (explicit double buffer); memset once
            # so padded view columns stay finite (mask zeroes them later).
            mg_buf0 = pp.tile([128, MGCAP * ROW12], BF16, tag="mgbuf0")
            mg_buf1 = pp.tile([128, MGCAP * ROW12], BF16, tag="mgbuf1")
            mg_buf2 = pp.tile([128, MGCAP * ROW12], BF16, tag="mgbuf2")
            mg_buf3 = pp.tile([128, MGCAP * ROW12], BF16, tag="mgbuf3")
            mg_bufs = [mg_buf0, mg_buf1, mg_buf2, mg_buf3]
            for _mb in mg_bufs:
                nc.vector.memset(_mb[:], 0.0)

            for li, lay in enumerate(layers):
                heads, hd = lay["heads"], lay["hd"]
                Fout, ncols, ROW = lay["Fout"], lay["ncols"], lay["row"]
                ktiles = lay["Fin"] // 128
                xts = [xT_a0, xT_a1][:ktiles] if li == 0 else \
                      ([xT_b] if li == 1 else [xT_a0])
                xt_next = xT_b if li == 0 else (xT_a0 if li == 1 else None)

                wsb = wp.tile([128, ktiles, ncols], BF16, tag="wsb")
                for kt in range(ktiles):
                    nc.sync.dma_start(wsb[:, kt, :], lay["W"][kt * 128 : (kt + 1) * 128, :])
                nc.sync.dma_start(bias_sb[:, 0:Fout], lay["brep"][:, 0:Fout])
                if lay["alr"] is not None:
                    al_sb = wp.tile([128, HID], BF16, tag="alsb")
                    nc.sync.dma_start(al_sb[:], lay["alr"][:])

                # ---- dense phase ----
                for cb in range(NBLK):
                    n0 = cb * 128
                    nn = min(128, L - n0)
                    ps = psp.tile([128, ncols], F32, tag="dps")
                    for kt in range(ktiles):
                        nc.tensor.matmul(
                            ps[0:nn, :], xts[kt][:, n0 : n0 + nn], wsb[:, kt, :],
                            start=(kt == 0), stop=(kt == ktiles - 1))
                    row_t = wp.tile([128, ROW], BF16, tag="rowt")
                    nc.vector.tensor_copy(row_t[0:nn, 0:Fout], ps[0:nn, 0:Fout])
                    if lay["alr"] is None:
                        elo = Fout  # bf16 col offset of el (fp32 pairs)
                        nc.vector.tensor_copy(
                            row_t[0:nn, elo : elo + 2 * heads].bitcast(F32),
                            ps[0:nn, Fout : Fout + heads])
                    nc.vector.tensor_copy(
                        er_all[0:nn, cb, 0:heads],
                        ps[0:nn, Fout + heads : Fout + 2 * heads])
                    nc.sync.dma_start(lay["tloc"][n0 : n0 + nn, :], row_t[0:nn, :])

                # ---- allgather ----
                nc.gpsimd.collective_compute(
                    "AllGather", OP.bypass,
                    replica_groups=[list(range(NC))],
                    ins=[lay["tloc"][:]], outs=[lay["tfull"][:]])

                TQ0 = lay["tfull"][0:ABOUND, :]
                TQ1 = lay["tfull"][ABOUND:N, :]


                # ---- edge phase (per group of blocks) ----
                for gi, (b0, nb, wag, wbg) in enumerate(groups):
                    W = wag + wbg
                    sA = soffs[gi]
                    mo = moffs[gi]
                    s16 = sum(8 * int(WAv[b0 + bi] + WBv[b0 + bi])
                              for bi in range(nb))

                    erb = er_all[:, b0 : b0 + nb, 0:heads]
                    lg = sp.tile([128, nb, W, heads], F32, tag="lg")

                    buf = mg_bufs[gi % 4]
                    mgv = buf[:, 0 : nb * W * ROW].rearrange(
                        "p (a w c) -> p a w c", a=nb, w=W, c=ROW)
                    for bi in range(nb):
                        wa = int(WAv[b0 + bi])
                        wb = int(WBv[b0 + bi])
                        offA, offB = boffs[gi][bi]
                        if wa:
                            nc.gpsimd.dma_gather(
                                mgv[:, bi, 0:wa, :], TQ0,
                                idx_sb[:, sA + offA : sA + offA + 8 * wa],
                                128 * wa, 128 * wa, ROW, single_packet=_SP,
                                queue_num=_gq[0] % 4)
                            _gq[0] += 1
                        if wb:
                            nc.gpsimd.dma_gather(
                                mgv[:, bi, wag : wag + wb, :], TQ1,
                                idx_sb[:, sA + offB : sA + offB + 8 * wb],
                                128 * wb, 128 * wb, ROW, single_packet=_SP,
                                queue_num=_gq[0] % 4)
                            _gq[0] += 1

                    # logits: el + er (broadcast over slots). For L1/L2 el is
                    # recomputed per edge from gathered h (rows carry h only);
                    # L3 keeps el as fp32 bits inside the row.
                    if lay["alr"] is not None:
                        tmp0 = tp.tile([128, nb, W, Fout], BF16, tag="tmp")
                        for bi in range(nb):
                            nc.vector.tensor_tensor(
                                tmp0[:, bi, :, :],
                                mgv[:, bi, :, 0:Fout],
                                al_sb[:, 0:Fout].unsqueeze(1).broadcast_to(
                                    [128, W, Fout]),
                                OP.mult)
                        elv = sp.tile([128, nb, W, heads], F32, tag="elv")
                        nc.vector.tensor_reduce(
                            elv[:].rearrange("p a w h -> p (a w) h"),
                            tmp0[:].rearrange("p a w (h d) -> p (a w) h d", h=heads, d=hd),
                            axis=AX.X, op=OP.add)
                        nc.vector.tensor_tensor(
                            lg[:], elv[:],
                            erb.unsqueeze(2).broadcast_to([128, nb, W, heads]),
                            OP.add)
                    else:
                        for bi in range(nb):
                            nc.vector.tensor_tensor(
                                lg[:, bi, :, :],
                                mgv[:, bi, :, Fout : Fout + 2 * heads].bitcast(F32),
                                erb[:, bi, :].unsqueeze(1).broadcast_to(
                                    [128, W, heads]),
                                OP.add)
                    # leaky relu: max(NEG*x, x)  (NEG < 1)
                    nc.vector.scalar_tensor_tensor(
                        lg[:], lg[:], NEG, lg[:], op0=OP.mult, op1=OP.max)
                    # clamp: stale el bits in masked pad slots can be huge;
                    # exp must stay finite so mask*exp stays 0 (not NaN)
                    nc.vector.scalar_tensor_tensor(
                        lg[:], lg[:], 30.0, lg[:], op0=OP.min, op1=OP.bypass)
                    ex = sp.tile([128, nb, W, heads], BF16, tag="ex")
                    nc.scalar.activation(ex[:], lg[:], AF.Exp)
                    # mask padding slots
                    mskv = msk_sb[:, mo : mo + nb * W].rearrange(
                        "p (a w) -> p a w", a=nb, w=W)
                    nc.vector.tensor_tensor(
                        ex[:], ex[:],
                        mskv.unsqueeze(3).broadcast_to([128, nb, W, heads]),
                        OP.mult)
                    # denominators
                    den = sp.tile([128, nb, heads], F32, tag="den")
                    nc.vector.tensor_reduce(
                        den[:], ex[:].rearrange("p a w h -> p a h w"),
                        axis=AX.X, op=OP.add)
                    nc.vector.scalar_tensor_tensor(
                        den[:], den[:], 1e-30, den[:], op0=OP.max, op1=OP.bypass)
                    rden = sp.tile([128, nb, heads], F32, tag="rden")
                    nc.vector.reciprocal(rden[:], den[:])

                    # weighted messages -> tmp [p, blk, W, Fout] (bf16), natural
                    # slot-major layout so every access is inner-contiguous
                    tmp = tp.tile([128, nb, W, Fout], BF16, tag="tmp")
                    _hrange = range(0) if _NODVE else range(heads)
                    for h in _hrange:
                        nc.vector.tensor_tensor(
                            tmp[:, :, :, h * hd : (h + 1) * hd],
                            mgv[:, :, :, h * hd : (h + 1) * hd],
                            ex[:, :, :, h].unsqueeze(3).broadcast_to(
                                [128, nb, W, hd]),
                            OP.mult)
                    # aggregate over slots: contiguous halving tree, final add
                    # lands in fp32
                    acc = wp.tile([128, nb, heads, hd], F32, tag="acc")
                    accf = acc[:].rearrange("p a h d -> p a (h d)")
                    w = 2 if _NODVE else W
                    nc.vector.memset(acc[:], 0.0) if _NODVE else None
                    while w > 2:
                        m = w // 2
                        nc.vector.tensor_tensor(
                            tmp[:, :, 0:m, :], tmp[:, :, 0:m, :],
                            tmp[:, :, w - m : w, :], OP.add)
                        w -= m
                    if _NODVE:
                        pass
                    elif w == 2:
                        nc.vector.tensor_tensor(
                            accf, tmp[:, :, 0, :], tmp[:, :, 1, :], OP.add)
                    else:
                        nc.vector.tensor_copy(accf, tmp[:, :, 0, :])
                    if DBG and gi == 0:
                        nc.sync.dma_start(
                            dbg_t[(li, "lg")][:],
                            lg[:].rearrange("p a w h -> p (a w h)"))
                        nc.sync.dma_start(
                            dbg_t[(li, "ex")][:],
                            ex[:].rearrange("p a w h -> p (a w h)"))
                        nc.sync.dma_start(
                            dbg_t[(li, "den")][:],
                            den[:].rearrange("p a h -> p (a h)"))
                        nc.sync.dma_start(
                            dbg_t[(li, "acc")][:],
                            acc[:].rearrange("p a h d -> p (a h d)"))
                        pass
                        nc.sync.dma_start(
                            dbg_t[(li, "tmp")][:],
                            tmp[:].rearrange("p a w f -> p (a w f)"))
                    # normalize + bias
                    nc.vector.tensor_tensor(
                        acc[:], acc[:],
                        rden[:].unsqueeze(3).broadcast_to([128, nb, heads, hd]),
                        OP.mult)
                    accf = acc[:].rearrange("p a h d -> p a (h d)")
                    nc.vector.tensor_tensor(
                        accf, accf,
                        bias_sb[:, 0:Fout].unsqueeze(1).broadcast_to(
                            [128, nb, Fout]),
                        OP.add)
                    if lay["relu"]:
                        nc.vector.scalar_tensor_tensor(
                        accf, accf, 0.0, accf, op0=OP.max, op1=OP.bypass)
                        for bi in range(nb):
                            pst = pspT.tile([128, 128], F32, tag="tps")
                            nc.tensor.transpose(pst[:], accf[:, bi, :], ident[:])
                            nc.vector.tensor_copy(
                                xt_next[:, (b0 + bi) * 128 : (b0 + bi + 1) * 128],
                                pst[:])
                    else:
                        nc.sync.dma_start(
                            out_ext[b0 * 128 : (b0 + nb) * 128, :].rearrange(
                                "(a q) f -> q a f", a=nb, q=128),
                            acc[:, :, 0, :])

    _split_multiwaits(nc)
    nc.compile()
    return nc


_CACHE = {}
LAST_EXEC_NS = None
LAST_RES = None


def kernel(feat, src, dst, W1, al1, ar1, b1, W2, al2, ar2, b2, W3, al3, ar3, b3):
    feat = np.asarray(feat, np.float32)
    key = (int(np.asarray(src[:100]).sum()), int(np.asarray(dst[:100]).sum()))
    if key in _CACHE:
        nc, meta, idx_alls, msk_alls = _CACHE[key]
    else:
        meta, idx_alls, msk_alls = _preprocess(src, dst)
        nc = _build_program(meta)
        _CACHE[key] = (nc, meta, idx_alls, msk_alls)

    node_order = meta["node_order"]

    W1e = _weights_ext(np.asarray(W1, np.float32), np.asarray(al1, np.float32),
                       np.asarray(ar1, np.float32), HEADS, HD)
    W2e = _weights_ext(np.asarray(W2, np.float32), np.asarray(al2, np.float32),
                       np.asarray(ar2, np.float32), HEADS, HD)
    W3e = _weights_ext(np.asarray(W3, np.float32), np.asarray(al3, np.float32),
                       np.asarray(ar3, np.float32), 1, OUT)
    assert W1e.shape[1] == 136 and W3e.shape[1] == 68

    ident = np.eye(128, dtype=np.float32)
    b1r = np.tile(np.asarray(b1, np.float32)[None, :], (128, 1))
    b2r = np.tile(np.asarray(b2, np.float32)[None, :], (128, 1))
    b3r = np.tile(np.asarray(b3, np.float32)[None, :], (128, 1))
    al1r = np.tile(np.asarray(al1, np.float32).reshape(1, -1), (128, 1)).astype(
        ml_dtypes.bfloat16)
    al2r = np.tile(np.asarray(al2, np.float32).reshape(1, -1), (128, 1)).astype(
        ml_dtypes.bfloat16)

    in_maps = []
    for c in range(NC):
        nodes = node_order[c * L : (c + 1) * L]
        featT_c = np.ascontiguousarray(feat[nodes, :].T).astype(ml_dtypes.bfloat16)
        in_maps.append(dict(
            featT=featT_c, W1e=W1e, W2e=W2e, W3e=W3e,
            b1r=b1r, b2r=b2r, b3r=b3r, al1r=al1r, al2r=al2r, ident=ident,
            idx_all=idx_alls[c], msk_all=np.asarray(msk_alls[c]),
        ))

    import os as _os
    _tdir = _os.environ.get("KERNEL_TRACE_DIR") or None
    res = run_bass_kernel_spmd(nc, in_maps, list(range(NC)), tmpdir=_tdir)
    global LAST_EXEC_NS, LAST_RES
    if res.exec_time_ns is not None:
        LAST_EXEC_NS = res.exec_time_ns
    LAST_RES = res

    out = np.empty((N, OUT), np.float32)
    for c in range(NC):
        nodes = node_order[c * L : (c + 1) * L]
        out[nodes] = res.results[c]["out"][0:L, :]
    return out



# revision 3
# speedup vs baseline: 1.3204x; 1.3204x over previous
"""GAT (3-layer DGL-style) on 8 Trainium2 NeuronCores.

Sharding: nodes partitioned across 8 cores (6250 each) by global degree-rank
snake assignment, relabeled within each core by a max-norm degree sort for
slot-grid uniformity. Edges sharded by dst core. Per layer: dense matmul
(bf16) produces per-node rows [h | el] (+ er kept in SBUF); AllGather
replicates the compact row table; a strided repack widens rows to a
512B-stride table so each gather descriptor fetches h AND el in one 512B
element (descriptor cost is per-descriptor, not per-byte). Each core then
runs the edge phase for its own dsts: per group-of-blocks dma_gather
(2 gathers: pass A/B over the int16-index split), batched 4D DVE ops for
softmax + in-place weighted-tree aggregation in a [dst-partition x slot]
layout.
"""

import numpy as np
import ml_dtypes

import concourse.bacc as bacc
import concourse.bass as bass
import concourse.mybir as mybir
from concourse import tile
from concourse._compat import cdiv
from concourse.bass_utils import run_bass_kernel_spmd
from bass_rust import SemaphoreHandle

N = 50000
E = 800000
NC = 8
L = N // NC              # 6250 nodes per core
NBLK = cdiv(L, 128)      # 49 dst blocks per core
HEADS = 4
HD = 32
HID = 128
OUT = 64
F0 = 256
NEG = 0.2
ABOUND = 5 * L           # nodes with new id < ABOUND are "pass A" (31250)
GROUP_COLS = 40          # slot-column budget per gather group
GROUP_MAXB = 5           # max blocks per gather group
import os as _os_pad
_SP = bool(_os_pad.environ.get("GAT_SP"))

F32 = mybir.dt.float32
BF16 = mybir.dt.bfloat16
I16 = mybir.dt.int16
AF = mybir.ActivationFunctionType
OP = mybir.AluOpType
AX = mybir.AxisListType


def _split_multiwaits(nc):
    nsplit = 0
    for bb in nc.main_func.blocks:
        i = 0
        while i < len(bb.instructions):
            ins = bb.instructions[i]
            si = ins.sync_info
            if si is not None and si.on_wait and len(si.on_wait) > 1:
                waits = list(si.on_wait)
                new_insts = []
                for w in waits[:-1]:
                    h = SemaphoreHandle(name=w.ant_name, num=w.id)
                    eng = nc.engines[ins.engine]
                    if w.wait_mode == "sem-ge-imm":
                        wi = eng.wait_ge(h, w.wait_value)
                    elif w.wait_mode == "sem-eq-imm":
                        wi = eng.wait_op(h, w.wait_value, "==")
                    else:
                        raise AssertionError(w.wait_mode)
                    removed = False
                    for b2 in nc.main_func.blocks:
                        if b2.instructions and b2.instructions[-1].name == wi.ins.name:
                            b2.instructions.pop()
                            removed = True
                            break
                    assert removed
                    new_insts.append(wi.ins)
                si.on_wait = [waits[-1]]
                for k, n in enumerate(new_insts):
                    bb.instructions.insert(i + k, n)
                i += len(new_insts)
                nsplit += 1
            i += 1
    return nsplit


def _cumcount(groups):
    """j-th occurrence index within each group (groups sorted)."""
    n = len(groups)
    if n == 0:
        return np.zeros(0, np.int64)
    first = np.r_[True, groups[1:] != groups[:-1]]
    idx = np.arange(n)
    start = idx[first]
    return idx - np.repeat(start, np.diff(np.r_[idx[first], n]))


def _wrap_idx(flat):
    """[nidx] stream -> [128, nidx//16] int16 wrapped index tile."""
    nidx = flat.shape[0]
    assert nidx % 128 == 0
    S = nidx // 16
    t = flat.reshape(S, 16).T.astype(np.int16)   # [16, S]
    return np.tile(t, (8, 1))                    # [128, S]


def _preprocess(src, dst):
    src = np.asarray(src, np.int64)
    dst = np.asarray(dst, np.int64)

    # global degree-rank snake assignment: rank r -> core r%8
    deg = np.bincount(dst, minlength=N)
    rank = np.argsort(-deg, kind="stable")
    core_of = np.empty(N, np.int64)
    core_of[rank] = np.arange(N) % NC

    half = core_of[src] >= 5        # pass B edges (src on cores 5-7)
    degA = np.bincount(dst[~half], minlength=N)
    degB = np.bincount(dst[half], minlength=N)

    perm = np.empty(N, np.int64)        # old id -> new id
    node_order = np.empty(N, np.int64)  # new id -> old id
    for c in range(NC):
        nodes = np.where(core_of == c)[0]
        order = np.lexsort((-degB[nodes],
                            -np.maximum(degA[nodes] * 4, degB[nodes] * 5)))
        node_order[c * L : (c + 1) * L] = nodes[order]
        perm[nodes[order]] = c * L + np.arange(L)

    nsrc = perm[src]
    ndst = perm[dst]
    epass = (nsrc >= ABOUND).astype(np.int64)

    cntA = np.bincount(ndst[epass == 0], minlength=N)
    cntB = np.bincount(ndst[epass == 1], minlength=N)

    # program-level W per (block, pass): max over cores
    WA = np.zeros(NBLK, np.int64)
    WB = np.zeros(NBLK, np.int64)
    for c in range(NC):
        la = np.zeros(NBLK * 128, np.int64)
        lb = np.zeros(NBLK * 128, np.int64)
        la[:L] = cntA[c * L : (c + 1) * L]
        lb[:L] = cntB[c * L : (c + 1) * L]
        WA = np.maximum(WA, la.reshape(NBLK, 128).max(1))
        WB = np.maximum(WB, lb.reshape(NBLK, 128).max(1))

    # adaptive grouping: uniform per-group VIEW widths (bounded footprint);
    # gathers stay tight per (block, pass) — padded view columns are masked.
    groups = []  # (b0, nb, WAg, WBg)
    b = 0
    while b < NBLK:
        nb = 1
        wag, wbg = int(WA[b]), int(WB[b])
        while b + nb < NBLK and nb < GROUP_MAXB:
            nwa = max(wag, int(WA[b + nb]))
            nwb = max(wbg, int(WB[b + nb]))
            if (nb + 1) * (nwa + nwb) > GROUP_COLS and nb >= 1:
                break
            wag, wbg = nwa, nwb
            nb += 1
        groups.append((b, nb, wag, wbg))
        b += nb

    # mask columns: group-major, block-major within group, [A slots | B slots]
    moffs = []
    Wtot = 0
    for (b0, nb, wag, wbg) in groups:
        moffs.append(Wtot)
        Wtot += nb * (wag + wbg)

    # idx stream offsets: tight per-(block, pass) pieces, A blocks then B
    soffs = []      # per group: start col
    boffs = []      # per group: per-block (offA, offB) within the group stream
    S16tot = 0
    for (b0, nb, wag, wbg) in groups:
        soffs.append(S16tot)
        per = []
        off = 0
        for bi in range(nb):
            per.append([off, 0])
            off += 8 * int(WA[b0 + bi])
        for bi in range(nb):
            per[bi][1] = off
            off += 8 * int(WB[b0 + bi])
        boffs.append([tuple(x) for x in per])
        S16tot += off

    idx_alls = []
    msk_alls = []
    for c in range(NC):
        m = (ndst // L) == c
        es = nsrc[m]
        ed = ndst[m] - c * L
        eq = epass[m]
        okey = ed * 2 + eq
        order = np.argsort(okey, kind="stable")
        es, ed, eq = es[order], ed[order], eq[order]
        j = _cumcount(okey[order])
        blk = ed // 128
        p = ed % 128

        msk = np.zeros((128, Wtot), np.float32)
        idx_pieces = []
        for gi, (b0, nb, wag, wbg) in enumerate(groups):
            for q, wq in ((0, WA), (1, WB)):
                for bi in range(nb):
                    w = int(wq[b0 + bi])
                    if w == 0:
                        continue
                    sel = (blk == b0 + bi) & (eq == q)
                    grid = np.zeros((128, w), np.int64)
                    v = es[sel] - (ABOUND if q else 0)
                    grid[p[sel], j[sel]] = v
                    idx_pieces.append(_wrap_idx(grid.T.reshape(-1)))

            mo = moffs[gi]
            W = wag + wbg
            in_g = (blk >= b0) & (blk < b0 + nb)
            bi_g = blk[in_g] - b0
            pg = p[in_g]
            jg = j[in_g]
            qg = eq[in_g]
            mgrid = msk[:, mo : mo + nb * W].reshape(128, nb, W)
            selA = qg == 0
            mgrid[pg[selA], bi_g[selA], jg[selA]] = 1.0
            selB = ~selA
            mgrid[pg[selB], bi_g[selB], wag + jg[selB]] = 1.0

        idx_alls.append(np.concatenate(idx_pieces, axis=1))
        msk_alls.append(msk.astype(ml_dtypes.bfloat16))

    assert idx_alls[0].shape[1] == S16tot
    meta = dict(groups=groups, moffs=moffs, soffs=soffs, boffs=boffs,
                WA=WA, WB=WB, Wtot=Wtot,
                S16tot=S16tot, node_order=node_order, perm=perm)
    return meta, idx_alls, msk_alls


def _weights_ext(W, al, ar, heads, hd):
    K = W.shape[0]
    Wr = W.reshape(K, heads, hd)
    A = np.einsum("khd,hd->kh", Wr, al).astype(np.float32)
    B = np.einsum("khd,hd->kh", Wr, ar).astype(np.float32)
    We = np.concatenate([W, A, B], axis=1).astype(np.float32)
    pad = (-We.shape[1]) % 4
    if pad:
        We = np.concatenate([We, np.zeros((K, pad), np.float32)], axis=1)
    return We.astype(ml_dtypes.bfloat16)


def _build_program(meta):
    groups = meta["groups"]
    moffs = meta["moffs"]
    soffs = meta["soffs"]
    boffs = meta["boffs"]
    WAv, WBv = meta["WA"], meta["WB"]
    S16tot = meta["S16tot"]
    Wtot = meta["Wtot"]
    MGCAP = max(nb * (wag + wbg) for (_b0, nb, wag, wbg) in groups)

    nc = bacc.Bacc("TRN2", num_swdge_queues=4)
    LP = NBLK * 128  # padded node count per core (6272)

    featT = nc.dram_tensor("featT", [F0, L], BF16, kind="ExternalInput")
    W1e = nc.dram_tensor("W1e", [F0, 136], BF16, kind="ExternalInput")
    W2e = nc.dram_tensor("W2e", [HID, 136], BF16, kind="ExternalInput")
    W3e = nc.dram_tensor("W3e", [HID, 68], BF16, kind="ExternalInput")
    b1r = nc.dram_tensor("b1r", [128, HID], F32, kind="ExternalInput")
    b2r = nc.dram_tensor("b2r", [128, HID], F32, kind="ExternalInput")
    b3r = nc.dram_tensor("b3r", [128, OUT], F32, kind="ExternalInput")
    ident_in = nc.dram_tensor("ident", [128, 128], F32, kind="ExternalInput")
    idx_in = nc.dram_tensor("idx_all", [128, S16tot], I16, kind="ExternalInput")
    msk_in = nc.dram_tensor("msk_all", [128, Wtot], BF16, kind="ExternalInput")
    out_ext = nc.dram_tensor("out", [LP, OUT], F32, kind="ExternalOutput")

    # compact row tables (AllGather payload) + 512B-stride gather tables
    tab_loc1 = nc.dram_tensor("tab_loc1", [L, 136], BF16)
    tab_loc2 = nc.dram_tensor("tab_loc2", [L, 136], BF16)
    tab_loc3 = nc.dram_tensor("tab_loc3", [L, 128], BF16)
    tab1 = nc.dram_tensor("tab1", [N, 136], BF16, addr_space="Shared")
    tab2 = nc.dram_tensor("tab2", [N, 136], BF16, addr_space="Shared")
    tab3 = nc.dram_tensor("tab3", [N, 128], BF16, addr_space="Shared")
    tab1w = nc.dram_tensor("tab1w", [N, 256], BF16)
    tab2w = nc.dram_tensor("tab2w", [N, 256], BF16)

    layers = [
        dict(Fin=F0, Fout=HID, heads=HEADS, hd=HD, W=W1e, ncols=136, row=256,
             tloc=tab_loc1, tfull=tab1, twide=tab1w, brep=b1r, relu=True),
        dict(Fin=HID, Fout=HID, heads=HEADS, hd=HD, W=W2e, ncols=136, row=256,
             tloc=tab_loc2, tfull=tab2, twide=tab2w, brep=b2r, relu=True),
        dict(Fin=HID, Fout=OUT, heads=1, hd=OUT, W=W3e, ncols=68, row=128,
             tloc=tab_loc3, tfull=tab3, twide=None, brep=b3r, relu=False),
    ]

    with tile.TileContext(nc) as tc:
        with (
            tc.tile_pool(name="persist", bufs=1) as pp,
            tc.tile_pool(name="work", bufs=2) as wp,
            tc.tile_pool(name="soft", bufs=4) as sp,
            tc.tile_pool(name="psum", bufs=2, space="PSUM") as psp,
            tc.tile_pool(name="psumT", bufs=2, space="PSUM") as pspT,
        ):
            idx_sb = pp.tile([128, S16tot], I16, tag="idx")
            nc.sync.dma_start(idx_sb[:], idx_in[:])
            msk_sb = pp.tile([128, Wtot], BF16, tag="msk")
            nc.sync.dma_start(msk_sb[:], msk_in[:])
            ident = pp.tile([128, 128], F32, tag="ident")
            nc.sync.dma_start(ident[:], ident_in[:])

            # xT buffers (features x nodes), bf16
            xT_a0 = pp.tile([128, LP], BF16, tag="xTa0")
            xT_a1 = pp.tile([128, LP], BF16, tag="xTa1")  # 2nd K-tile (layer 0)
            xT_b = pp.tile([128, LP], BF16, tag="xTb")
            nc.sync.dma_start(xT_a0[:, 0:L], featT[0:128, :])
            nc.sync.dma_start(xT_a1[:, 0:L], featT[128:256, :])

            _gq = [0]  # gather queue round-robin counter
            er_all0 = pp.tile([128, NBLK, HEADS], F32, tag="er0")
            er_all1 = pp.tile([128, NBLK, HEADS], F32, tag="er1")
            nc.vector.memset(er_all0[:], 0.0)
            nc.vector.memset(er_all1[:], 0.0)
            er_alls = [er_all0, er_all1]
            bias_sb = pp.tile([128, HID], F32, tag="bias")

            # persistent gather buffers (explicit multi-buffer); memset once
            # so stale slot columns stay finite (mask zeroes them later).
            mg_bufs = []
            for i in range(4):
                mgb = pp.tile([128, MGCAP * 256], BF16, tag=f"mgbuf{i}")
                mg_bufs.append(mgb)
                nc.vector.memset(mgb[:], 0.0)

            for li, lay in enumerate(layers):
                heads, hd = lay["heads"], lay["hd"]
                Fout, ncols, ROW = lay["Fout"], lay["ncols"], lay["row"]
                ktiles = lay["Fin"] // 128
                xts = [xT_a0, xT_a1][:ktiles] if li == 0 else \
                      ([xT_b] if li == 1 else [xT_a0])
                xt_next = xT_b if li == 0 else (xT_a0 if li == 1 else None)
                er_all = er_alls[li % 2]
                NROWC = 136 if li < 2 else 128   # compact row width

                wsb = wp.tile([128, ktiles, ncols], BF16, tag="wsb")
                for kt in range(ktiles):
                    nc.sync.dma_start(wsb[:, kt, :], lay["W"][kt * 128 : (kt + 1) * 128, :])
                nc.sync.dma_start(bias_sb[:, 0:Fout], lay["brep"][:, 0:Fout])

                # ---- dense phase ----
                for cb in range(NBLK):
                    n0 = cb * 128
                    nn = min(128, L - n0)
                    ps = psp.tile([128, ncols], F32, tag="dps")
                    for kt in range(ktiles):
                        nc.tensor.matmul(
                            ps[0:nn, :], xts[kt][:, n0 : n0 + nn], wsb[:, kt, :],
                            start=(kt == 0), stop=(kt == ktiles - 1))
                    row_t = wp.tile([128, NROWC], BF16, tag="rowt")
                    nc.vector.tensor_copy(row_t[0:nn, 0:Fout], ps[0:nn, 0:Fout])
                    # el packed as fp32 bit pairs right after h
                    nc.vector.tensor_copy(
                        row_t[0:nn, Fout : Fout + 2 * heads].bitcast(F32),
                        ps[0:nn, Fout : Fout + heads])
                    nc.vector.tensor_copy(
                        er_all[0:nn, cb, 0:heads],
                        ps[0:nn, Fout + heads : Fout + 2 * heads])
                    nc.sync.dma_start(lay["tloc"][n0 : n0 + nn, :], row_t[0:nn, :])

                # ---- allgather (compact rows) ----
                nc.gpsimd.collective_compute(
                    "AllGather", OP.bypass,
                    replica_groups=[list(range(NC))],
                    ins=[lay["tloc"][:]], outs=[lay["tfull"][:]])

                # ---- widen to 512B-stride gather table (chunked) ----
                if lay["twide"] is not None:
                    NCH = 8
                    rows = cdiv(N, NCH)
                    for ch in range(NCH):
                        r0 = ch * rows
                        r1 = min(N, r0 + rows)
                        nc.sync.dma_start(
                            lay["twide"][r0:r1, 0:136], lay["tfull"][r0:r1, :])
                    gtab = lay["twide"]
                else:
                    gtab = lay["tfull"]

                TQ0 = gtab[0:ABOUND, :]
                TQ1 = gtab[ABOUND:N, :]

                # L3 reads el at a different slot alignment (256B slots), so
                # stale bytes there may be L1/L2's undefined wide-table pad
                # columns — zero the gather buffers so stale el reads are 0.
                if li == 2:
                    for mgb in mg_bufs:
                        nc.vector.memset(mgb[:], 0.0)

                # ---- edge phase (per group of blocks) ----
                for gi, (b0, nb, wag, wbg) in enumerate(groups):
                    W = wag + wbg
                    sA = soffs[gi]
                    mo = moffs[gi]

                    erb = er_all[:, b0 : b0 + nb, 0:heads]
                    lg = sp.tile([128, nb, W, heads], F32, tag="lg")

                    buf = mg_bufs[gi % 4]
                    mgv = buf[:, 0 : nb * W * ROW].rearrange(
                        "p (a w c) -> p a w c", a=nb, w=W, c=ROW)
                    for bi in range(nb):
                        wa = int(WAv[b0 + bi])
                        wb = int(WBv[b0 + bi])
                        offA, offB = boffs[gi][bi]
                        if wa:
                            nc.gpsimd.dma_gather(
                                mgv[:, bi, 0:wa, :], TQ0,
                                idx_sb[:, sA + offA : sA + offA + 8 * wa],
                                128 * wa, 128 * wa, ROW, single_packet=_SP,
                                queue_num=_gq[0] % 4)
                            _gq[0] += 1
                        if wb:
                            nc.gpsimd.dma_gather(
                                mgv[:, bi, wag : wag + wb, :], TQ1,
                                idx_sb[:, sA + offB : sA + offB + 8 * wb],
                                128 * wb, 128 * wb, ROW, single_packet=_SP,
                                queue_num=_gq[0] % 4)
                            _gq[0] += 1

                    # logits: el (gathered, fp32 bits in the row) + er
                    nc.vector.tensor_tensor(
                        lg[:],
                        mgv[:, :, :, Fout : Fout + 2 * heads].bitcast(F32),
                        erb.unsqueeze(2).broadcast_to([128, nb, W, heads]),
                        OP.add)
                    # leaky relu: max(NEG*x, x)  (NEG < 1)
                    nc.vector.scalar_tensor_tensor(
                        lg[:], lg[:], NEG, lg[:], op0=OP.mult, op1=OP.max)
                    # clamp: stale el bits in masked pad slots can be huge;
                    # exp must stay finite so mask*exp stays 0 (not NaN)
                    nc.vector.scalar_tensor_tensor(
                        lg[:], lg[:], 30.0, lg[:], op0=OP.min, op1=OP.bypass)
                    ex = sp.tile([128, nb, W, heads], BF16, tag="ex")
                    nc.scalar.activation(ex[:], lg[:], AF.Exp)
                    # mask padding slots
                    mskv = msk_sb[:, mo : mo + nb * W].rearrange(
                        "p (a w) -> p a w", a=nb, w=W)
                    nc.vector.tensor_tensor(
                        ex[:], ex[:],
                        mskv.unsqueeze(3).broadcast_to([128, nb, W, heads]),
                        OP.mult)
                    # denominators
                    den = sp.tile([128, nb, heads], F32, tag="den")
                    nc.vector.tensor_reduce(
                        den[:], ex[:].rearrange("p a w h -> p a h w"),
                        axis=AX.X, op=OP.add)
                    nc.vector.scalar_tensor_tensor(
                        den[:], den[:], 1e-30, den[:], op0=OP.max, op1=OP.bypass)
                    rden = sp.tile([128, nb, heads], F32, tag="rden")
                    nc.vector.reciprocal(rden[:], den[:])

                    # weighted messages IN PLACE over the gathered h columns
                    for h in range(heads):
                        nc.vector.tensor_tensor(
                            mgv[:, :, :, h * hd : (h + 1) * hd],
                            mgv[:, :, :, h * hd : (h + 1) * hd],
                            ex[:, :, :, h].unsqueeze(3).broadcast_to(
                                [128, nb, W, hd]),
                            OP.mult)
                    # aggregate over slots: contiguous halving tree, final add
                    # lands in fp32
                    acc = wp.tile([128, nb, heads, hd], F32, tag="acc")
                    accf = acc[:].rearrange("p a h d -> p a (h d)")
                    w = W
                    while w > 2:
                        m = w // 2
                        nc.vector.tensor_tensor(
                            mgv[:, :, 0:m, 0:Fout], mgv[:, :, 0:m, 0:Fout],
                            mgv[:, :, w - m : w, 0:Fout], OP.add)
                        w -= m
                    if w == 2:
                        nc.vector.tensor_tensor(
                            accf, mgv[:, :, 0, 0:Fout], mgv[:, :, 1, 0:Fout],
                            OP.add)
                    else:
                        nc.vector.tensor_copy(accf, mgv[:, :, 0, 0:Fout])
                    # normalize + bias
                    nc.vector.tensor_tensor(
                        acc[:], acc[:],
                        rden[:].unsqueeze(3).broadcast_to([128, nb, heads, hd]),
                        OP.mult)
                    nc.vector.tensor_tensor(
                        accf, accf,
                        bias_sb[:, 0:Fout].unsqueeze(1).broadcast_to(
                            [128, nb, Fout]),
                        OP.add)
                    if lay["relu"]:
                        nc.vector.scalar_tensor_tensor(
                            accf, accf, 0.0, accf, op0=OP.max, op1=OP.bypass)
                        for bi in range(nb):
                            pst = pspT.tile([128, 128], F32, tag="tps")
                            nc.tensor.transpose(pst[:], accf[:, bi, :], ident[:])
                            nc.vector.tensor_copy(
                                xt_next[:, (b0 + bi) * 128 : (b0 + bi + 1) * 128],
                                pst[:])
                    else:
                        nc.sync.dma_start(
                            out_ext[b0 * 128 : (b0 + nb) * 128, :].rearrange(
                                "(a q) f -> q a f", a=nb, q=128),
                            acc[:, :, 0, :])

    _split_multiwaits(nc)
    nc.compile()
    return nc


_CACHE = {}
LAST_EXEC_NS = None
LAST_RES = None


def kernel(feat, src, dst, W1, al1, ar1, b1, W2, al2, ar2, b2, W3, al3, ar3, b3):
    feat = np.asarray(feat, np.float32)
    key = (int(np.asarray(src[:100]).sum()), int(np.asarray(dst[:100]).sum()))
    if key in _CACHE:
        nc, meta, idx_alls, msk_alls = _CACHE[key]
    else:
        meta, idx_alls, msk_alls = _preprocess(src, dst)
        nc = _build_program(meta)
        _CACHE[key] = (nc, meta, idx_alls, msk_alls)

    node_order = meta["node_order"]

    W1e = _weights_ext(np.asarray(W1, np.float32), np.asarray(al1, np.float32),
                       np.asarray(ar1, np.float32), HEADS, HD)
    W2e = _weights_ext(np.asarray(W2, np.float32), np.asarray(al2, np.float32),
                       np.asarray(ar2, np.float32), HEADS, HD)
    W3e = _weights_ext(np.asarray(W3, np.float32), np.asarray(al3, np.float32),
                       np.asarray(ar3, np.float32), 1, OUT)
    assert W1e.shape[1] == 136 and W3e.shape[1] == 68

    ident = np.eye(128, dtype=np.float32)
    b1r = np.tile(np.asarray(b1, np.float32)[None, :], (128, 1))
    b2r = np.tile(np.asarray(b2, np.float32)[None, :], (128, 1))
    b3r = np.tile(np.asarray(b3, np.float32)[None, :], (128, 1))

    in_maps = []
    for c in range(NC):
        nodes = node_order[c * L : (c + 1) * L]
        featT_c = np.ascontiguousarray(feat[nodes, :].T).astype(ml_dtypes.bfloat16)
        in_maps.append(dict(
            featT=featT_c, W1e=W1e, W2e=W2e, W3e=W3e,
            b1r=b1r, b2r=b2r, b3r=b3r, ident=ident,
            idx_all=idx_alls[c], msk_all=np.asarray(msk_alls[c]),
        ))

    import os as _os
    _tdir = _os.environ.get("KERNEL_TRACE_DIR") or None
    res = run_bass_kernel_spmd(nc, in_maps, list(range(NC)), tmpdir=_tdir)
    global LAST_EXEC_NS, LAST_RES
    if res.exec_time_ns is not None:
        LAST_EXEC_NS = res.exec_time_ns
    LAST_RES = res

    out = np.empty((N, OUT), np.float32)
    for c in range(NC):
        nodes = node_order[c * L : (c + 1) * L]
        out[nodes] = res.results[c]["out"][0:L, :]
    return out
